# revision 12
# baseline (speedup 1.0000x reference)
"""BWGNN (Bernstein-basis spectral GNN) on 8 Trainium2 NeuronCores.

Math (equivalent to the reference):
    h  = relu(relu(X W1 + b1) W2 + b2)
    L f = f - D^-1/2 A D^-1/2 f        (A via segment-sum of src->dst edges)
    g1 = L h ; g2 = L g1
    out = relu([h|g1|g2] @ W3' + b3) @ W4 + b4
W3' folds the constant Bernstein theta coefficients into W3 (a
compile-time linear reparam of the concat-of-filters formulation).

Distribution: nodes sharded 8 ways (12500/core, padded to 12544 = 98
buckets x 128, degree-sorted within core). Per Laplacian hop:
  - tables of f*dinv (bf16 rows duplicated to 256B) are AllGathered in
    4 source-position chunks so every core holds all rows; each chunk
    is one int16-addressable gather window,
  - each core gathers its in-edge messages with dma_gather (int16 idx,
    one window per SWDGE queue, 49 groups of 2 dst buckets),
  - segment-sum per 128-dst-node bucket via one-hot matmuls into PSUM.
The schedule keeps the 4 SWDGE queues (the ~10ns/descriptor critical
resource) continuously busy: gathers are pre-issued 6 groups deep per
queue, PSUM eviction happens on the Scalar engine (activation COPY with
a dinv scale), and all remaining epilogue work is issued 2 groups late
so the in-order Vector/Tensor queues never stall ahead of independent
one-hot builds / matmuls.
Dense layers are data-parallel per bucket, feature-major, bf16 matmuls
with fp32 PSUM/pointwise.
"""
import os
import sys
import types

import numpy as np
import ml_dtypes

import concourse.bacc as bacc
import concourse.bass as bass
import concourse.mybir as mybir
import concourse.tile as tile
from concourse.bass_utils import run_bass_kernel_spmd


# --- antenv.axon_hooks shim (the agent image lacks it; needed only when
# NTFF tracing is requested) ---------------------------------------------
def _install_axon_shim():
    if "antenv.axon_hooks" in sys.modules:
        return
    state = {"hook": None}
    mod = types.ModuleType("antenv.axon_hooks")
    mod.set_axon_ntff_profile_hook = lambda h: state.__setitem__("hook", h)
    mod.get_axon_ntff_profile_hook = lambda: state["hook"]
    sys.modules["antenv.axon_hooks"] = mod
    try:
        import antenv

        antenv.axon_hooks = mod
    except Exception:
        pass
    try:
        from trn_agent_boot.trn_boot import _ntff_profile_via_ctypes

        h = _ntff_profile_via_ctypes("/opt/axon/libaxon_pjrt.so")
        if h is not None:
            mod.set_axon_ntff_profile_hook(h)
    except Exception:
        pass


_install_axon_shim()


# --- BIR fixup: this walrus build rejects >1 sync wait per instruction;
# move excess waits onto preceding InstNoOp carriers (same engine, so
# semantics are unchanged) ------------------------------------------------
def split_waits(nc, max_waits=1):
    for f in nc.m.functions:
        for blk in f.blocks:
            new_insts = []
            for inst in blk.instructions:
                si = inst.sync_info
                if si is not None and len(si.on_wait) > max_waits:
                    waits = list(si.on_wait)
                    extra, keep = waits[:-max_waits], waits[-max_waits:]
                    for i in range(0, len(extra), max_waits):
                        nop = mybir.InstNoOp(
                            name=nc.get_next_instruction_name(), ins=[], outs=[]
                        )
                        nop.engine = inst.engine
                        nop.sync_info = mybir.SyncInfo(
                            on_wait=extra[i : i + max_waits], on_update=[]
                        )
                        nc.register_instruction(nop)
                        new_insts.append(nop)
                    si.on_wait = keep
                new_insts.append(inst)
            blk.instructions[:] = new_insts

N = 100_000
E = 3_200_000
F = 64
C = 2
THETAS = np.array([[3.0, -3.0, 0.75], [0.0, 3.0, -1.5], [0.0, 0.0, 0.75]])
W = 8
RPC = 12500
R = 12544                  # 98 buckets x 128
NB = 98
GROUP = 2                  # dst buckets per gather group
NG = NB // GROUP           # 49 groups
NQUEUE = 4
F2 = 2 * F                 # duplicated bf16 row = 256B

# 4 source-position chunks (for chunked AllGather + int16 gather windows)
CHB = [25, 25, 24, 24]                      # buckets per chunk
CHR = [b * 128 for b in CHB]                # rows per chunk
CHO = [0, 3200, 6400, 9472]                 # row offsets
WS = [8 * r for r in CHR]                   # gather window sizes (all < 32768)
CH_LAST_BUCKET = [24, 49, 73, 97]           # last dst bucket index per chunk
PRE = 4                                     # gather pre-issue depth (groups)
DELAY = 2                                   # epilogue flush lag (groups)

DT_BF16 = mybir.dt.bfloat16
DT_F32 = mybir.dt.float32
DT_I16 = mybir.dt.int16

LAST_EXEC_NS = None
_TRACE = os.environ.get("BWGNN_TRACE", "0") == "1"


def _preprocess(src, dst):
    src = np.asarray(src).astype(np.int64).ravel()
    dst = np.asarray(dst).astype(np.int64).ravel()
    deg = np.bincount(dst, minlength=N)
    dinv = (np.clip(deg, 1, None).astype(np.float64) ** -0.5).astype(np.float32)

    pos = np.empty(N, dtype=np.int64)
    perm = np.full((W, R), -1, dtype=np.int64)
    for c in range(W):
        g0 = c * RPC
        order = np.argsort(-deg[g0 : g0 + RPC], kind="stable")
        perm[c, :RPC] = g0 + order
        pos[g0 + order] = np.arange(RPC)

    e_c = dst // RPC
    e_pos = pos[dst]
    e_b = e_pos // 128
    e_p = (e_pos % 128).astype(np.float32)
    e_g = e_b // GROUP
    # chunked-AllGather table layout: window q = chunk of src position;
    # offset = srccore * CHR[q] + (srcpos - CHO[q])
    s_c = src // RPC
    s_pos = pos[src]
    e_q = np.searchsorted(np.array(CHO[1:] + [R]), s_pos, side="right")
    e_off = s_c * np.array(CHR)[e_q] + (s_pos - np.array(CHO)[e_q])

    key_cbq = (e_c * NB + e_b) * 4 + e_q
    cnt = np.bincount(key_cbq, minlength=W * NB * 4).reshape(W, NB, 4)
    tcnt_bq = np.maximum(1, np.ceil(cnt.max(axis=0) / 128.0).astype(np.int64))

    colof = np.zeros((NG, 4), dtype=np.int64)
    tileof = np.zeros((NG, 4), dtype=np.int64)
    nidx = np.zeros((NG, 4), dtype=np.int64)
    col = til = 0
    for g in range(NG):
        for q in range(4):
            t = int(tcnt_bq[g * GROUP : (g + 1) * GROUP, q].sum())
            colof[g, q], tileof[g, q], nidx[g, q] = col, til, t * 128
            col += t * 8
            til += t
    COLS, NTILES = col, til

    sort_key = ((e_c * NG + e_g) * 4 + e_q) * NB + e_b
    eorder = np.argsort(sort_key, kind="stable")

    idx16 = np.zeros((W, 128, COLS), dtype=np.int16)
    dstl = np.full((W, 128, NTILES), -1.0, dtype=np.float32)

    for c in range(W):
        eo = eorder[e_c[eorder] == c]
        ptr = 0
        for g in range(NG):
            for q in range(4):
                n = int(nidx[g, q])
                flat_off = np.zeros(n, dtype=np.int64)
                flat_dst = np.full(n, -1.0, dtype=np.float32)
                fpos = 0
                for b in range(g * GROUP, (g + 1) * GROUP):
                    m = int(cnt[c, b, q])
                    seg = eo[ptr : ptr + m]
                    ptr += m
                    flat_off[fpos : fpos + m] = e_off[seg]
                    flat_dst[fpos : fpos + m] = e_p[seg]
                    fpos += int(tcnt_bq[b, q]) * 128
                wrapped = flat_off.reshape(-1, 16).T.astype(np.int16)
                c0 = int(colof[g, q])
                idx16[c, :, c0 : c0 + n // 16] = np.tile(wrapped, (8, 1))
                t0 = int(tileof[g, q])
                dstl[c, :, t0 : t0 + n // 128] = flat_dst.reshape(-1, 128).T
        assert ptr == int((e_c == c).sum())

    return dict(
        dinv=dinv,
        perm=perm,
        tcnt_bq=tcnt_bq,
        colof=colof,
        tileof=tileof,
        nidx=nidx,
        COLS=COLS,
        NTILES=NTILES,
        idx16=idx16,
        dstl=dstl.astype(ml_dtypes.bfloat16),
    )


def _build(meta):
    tcnt_bq, colof, tileof, nidx = (
        meta["tcnt_bq"],
        meta["colof"],
        meta["tileof"],
        meta["nidx"],
    )
    COLS, NTILES = meta["COLS"], meta["NTILES"]

    nc = bacc.Bacc(None, target_bir_lowering=False, num_swdge_queues=NQUEUE, dynamic_dma_scratch_size=32768)

    xT = nc.declare_dram_parameter("xT", [F, R], DT_BF16, isOutput=False)
    dinv_in = nc.declare_dram_parameter("dinv", [128, NB], DT_F32, isOutput=False)
    w1 = nc.declare_dram_parameter("w1", [F, F], DT_BF16, isOutput=False)
    w2 = nc.declare_dram_parameter("w2", [F, F], DT_BF16, isOutput=False)
    w3p = nc.declare_dram_parameter("w3p", [F, 3 * F], DT_BF16, isOutput=False)
    w4 = nc.declare_dram_parameter("w4", [F, C], DT_BF16, isOutput=False)
    b1 = nc.declare_dram_parameter("b1", [F, 1], DT_F32, isOutput=False)
    b2 = nc.declare_dram_parameter("b2", [F, 1], DT_F32, isOutput=False)
    b3 = nc.declare_dram_parameter("b3", [F, 1], DT_F32, isOutput=False)
    b4 = nc.declare_dram_parameter("b4", [C, 1], DT_F32, isOutput=False)
    ident = nc.declare_dram_parameter("ident", [128, 128], DT_F32, isOutput=False)
    identb = nc.declare_dram_parameter("identb", [128, 128], DT_BF16, isOutput=False)
    iota4 = nc.declare_dram_parameter("iota4", [128, 8 * 128], DT_BF16, isOutput=False)
    idx_in = nc.declare_dram_parameter("idx", [128, COLS], DT_I16, isOutput=False)
    dstl_in = nc.declare_dram_parameter("dstl", [128, NTILES], DT_BF16, isOutput=False)
    outT = nc.declare_dram_parameter("outT", [NB, C, 128], DT_F32, isOutput=True)

    cc1_in = nc.dram_tensor("cc1_in", [R, F2], DT_BF16)
    cc2_in = nc.dram_tensor("cc2_in", [R, F2], DT_BF16)
    t1 = [
        nc.dram_tensor(f"t1_{k}", [WS[k], F2], DT_BF16, addr_space="Shared")
        for k in range(4)
    ]
    t2 = [
        nc.dram_tensor(f"t2_{k}", [WS[k], F2], DT_BF16, addr_space="Shared")
        for k in range(4)
    ]
    rg = [list(range(W))]
    RELU = mybir.ActivationFunctionType.Relu
    COPY = mybir.ActivationFunctionType.Copy

    def ag_chunk(cc_in, tabs, k):
        nc.gpsimd.collective_compute(
            "AllGather",
            mybir.AluOpType.bypass,
            replica_groups=rg,
            ins=[cc_in[CHO[k] : CHO[k] + CHR[k], :]],
            outs=[tabs[k][:]],
        )

    with tile.TileContext(nc) as tc:
        with (
            tc.tile_pool(name="const", bufs=1) as constp,
            tc.tile_pool(name="persist", bufs=1) as persist,
        ):
            def cload(nm, shape, dtype, srcap):
                t = constp.tile(shape, dtype, name=nm, tag=nm)
                nc.sync.dma_start(out=t[:], in_=srcap)
                return t

            w1s = cload("w1s", [F, F], DT_BF16, w1[:])
            w2s = cload("w2s", [F, F], DT_BF16, w2[:])
            w3s = cload("w3s", [F, 3 * F], DT_BF16, w3p[:])
            w4s = cload("w4s", [F, C], DT_BF16, w4[:])
            b1s = cload("b1s", [F, 1], DT_F32, b1[:])
            b2s = cload("b2s", [F, 1], DT_F32, b2[:])
            b3s = cload("b3s", [F, 1], DT_F32, b3[:])
            b4s = cload("b4s", [C, 1], DT_F32, b4[:])
            idents = cload("idents", [128, 128], DT_F32, ident[:])
            identbs = cload("identbs", [128, 128], DT_BF16, identb[:])
            iotas = cload("iotas", [128, 8 * 128], DT_BF16, iota4[:])
            dinvs = cload("dinvs", [128, NB], DT_F32, dinv_in[:])
            dstls = cload("dstls", [128, NTILES], DT_BF16, dstl_in[:])

            h_node = persist.tile([128, NB * F], DT_BF16)
            g1_node = persist.tile([128, NB * F], DT_BF16)

            # ---------------- phase A ----------------
            with (
                tc.tile_pool(name="pA", bufs=3) as pA,
                tc.tile_pool(name="psA", bufs=2, space="PSUM") as psA,
            ):
                for bp in range(NB // 2):
                    b0 = 2 * bp
                    xt = pA.tile([F, 256], DT_BF16, tag="xt")
                    nc.sync.dma_start(out=xt[:], in_=xT[:, b0 * 128 : b0 * 128 + 256])
                    ps1 = psA.tile([F, 256], DT_F32, tag="ps1")
                    nc.tensor.matmul(ps1[:], w1s[:], xt[:], start=True, stop=True)
                    h1 = pA.tile([F, 256], DT_BF16, tag="h1")
                    nc.vector.tensor_scalar(
                        out=h1[:], in0=ps1[:], scalar1=b1s[:], scalar2=0.0,
                        op0=mybir.AluOpType.add, op1=mybir.AluOpType.max,
                    )
                    ps2 = psA.tile([F, 256], DT_F32, tag="ps2")
                    nc.tensor.matmul(ps2[:], w2s[:], h1[:], start=True, stop=True)
                    h2T = pA.tile([F, 256], DT_F32, tag="h2T")
                    nc.scalar.activation(h2T[:], ps2[:], RELU, bias=b2s[:])
                    for bi in range(2):
                        b = b0 + bi
                        psT = psA.tile([128, F], DT_F32, tag="psT")
                        nc.tensor.transpose(
                            psT[:], h2T[:, bi * 128 : (bi + 1) * 128], idents[:F, :F]
                        )
                        nc.vector.tensor_copy(
                            out=h_node[:, b * F : (b + 1) * F], in_=psT[:]
                        )
                        tw = pA.tile([128, F], DT_BF16, tag="tw")
                        nc.vector.tensor_scalar(
                            out=tw[:],
                            in0=psT[:],
                            scalar1=dinvs[:, b : b + 1],
                            scalar2=None,
                            op0=mybir.AluOpType.mult,
                        )
                        nc.scalar.dma_start(
                            out=cc1_in[b * 128 : (b + 1) * 128, 0:F], in_=tw[:]
                        )
                        if b == CH_LAST_BUCKET[0]:
                            ag_chunk(cc1_in, t1, 0)

            # ---------------- hops: merged cross-hop pipeline ----------------
            def epi_B(b, t1e, epi, psE):
                # g1 = h - agg*dinv   (Vector; inputs ready - issued late)
                nc.vector.tensor_tensor(
                    out=g1_node[:, b * F : (b + 1) * F],
                    in0=h_node[:, b * F : (b + 1) * F],
                    in1=t1e[:],
                    op=mybir.AluOpType.subtract,
                )
                tw = epi.tile([128, F], DT_BF16, tag="tw2", name="tw2", bufs=4)
                nc.scalar.activation(
                    tw[:],
                    g1_node[:, b * F : (b + 1) * F],
                    COPY,
                    scale=dinvs[:, b : b + 1],
                )
                nc.scalar.dma_start(
                    out=cc2_in[b * 128 : (b + 1) * 128, 0:F], in_=tw[:]
                )
                for k in range(3):
                    if b == CH_LAST_BUCKET[k]:
                        ag_chunk(cc2_in, t2, k)

            def epi_C(b, t2e, epi, psE):
                g2n = epi.tile([128, F], DT_BF16, tag="g2n", name="g2n", bufs=4)
                nc.vector.tensor_tensor(
                    out=g2n[:],
                    in0=g1_node[:, b * F : (b + 1) * F],
                    in1=t2e[:],
                    op=mybir.AluOpType.subtract,
                )
                psZ = psE.tile([F, 128], DT_F32, tag="psZ")
                pst = psE.tile([F, 128], DT_BF16, tag="pst")
                fT = epi.tile([F, 128], DT_BF16, tag="fT", bufs=2)
                nc.tensor.transpose(
                    pst[:], h_node[:, b * F : (b + 1) * F], identbs[:]
                )
                nc.scalar.activation(fT[:], pst[:], COPY)
                nc.tensor.matmul(
                    psZ[:], w3s[:, 0:F], fT[:], start=True, stop=False
                )
                g1T = epi.tile([F, 128], DT_BF16, tag="g1T", bufs=2)
                nc.tensor.transpose(
                    pst[:], g1_node[:, b * F : (b + 1) * F], identbs[:]
                )
                nc.scalar.activation(g1T[:], pst[:], COPY)
                nc.tensor.matmul(
                    psZ[:], w3s[:, F : 2 * F], g1T[:], start=False, stop=False
                )
                g2T = epi.tile([F, 128], DT_BF16, tag="g2T", bufs=2)
                nc.tensor.transpose(pst[:], g2n[:], identbs[:])
                nc.scalar.activation(g2T[:], pst[:], COPY)
                nc.tensor.matmul(
                    psZ[:], w3s[:, 2 * F : 3 * F], g2T[:], start=False, stop=True
                )
                zb = epi.tile([F, 128], DT_BF16, tag="zb", bufs=2)
                nc.scalar.activation(zb[:], psZ[:], RELU, bias=b3s[:])
                psO = psE.tile([C, 128], DT_F32, tag="psO")
                nc.tensor.matmul(psO[:], w4s[:], zb[:], start=True, stop=True)
                oadd = epi.tile([C, 128], DT_F32, tag="oadd", bufs=2)
                nc.vector.tensor_scalar(
                    out=oadd[:],
                    in0=psO[:],
                    scalar1=b4s[:],
                    scalar2=None,
                    op0=mybir.AluOpType.add,
                )
                nc.scalar.dma_start(out=outT[b, :, :], in_=oadd[:])

            with (
                tc.tile_pool(name="ixp", bufs=5) as ixp,
                tc.tile_pool(name="gpl", bufs=5) as gpool,
                tc.tile_pool(name="ohp", bufs=4) as ohp,
                tc.tile_pool(name="psB", bufs=2, space="PSUM") as psB,
                tc.tile_pool(name="epi", bufs=8) as epi,
                tc.tile_pool(name="psE", bufs=2, space="PSUM") as psE,
            ):
                gts = {}
                pending = []
                TABS = [t1, t2]
                EPIS = [epi_B, epi_C]

                def issue_gather(hh, g, q, prolog=False):
                    n = int(nidx[g, q])
                    c0 = int(colof[g, q])
                    ix = ixp.tile(
                        [128, n // 16], DT_I16, tag=f"ix{q}", name=f"ix{hh}_{g}_{q}"
                    )
                    ix_eng = nc.gpsimd if prolog else nc.sync
                    ix_eng.dma_start(out=ix[:], in_=idx_in[:, c0 : c0 + n // 16])
                    gt = gpool.tile(
                        [128, (n // 128) * F2], DT_BF16, tag=f"g{q}",
                        name=f"gt{hh}_{g}_{q}",
                    )
                    nc.gpsimd.dma_gather(
                        gt[:].rearrange("p (c f) -> p c f", f=F2),
                        TABS[hh][q][:, :],
                        ix[:],
                        num_idxs=n,
                        num_idxs_reg=n,
                        elem_size=F2,
                        single_packet=False,
                        queue_num=(q + g) % 4,
                    )
                    gts[(hh, g, q)] = gt

                def issue_mm(hh, g):
                    tloc = {q: 0 for q in range(4)}
                    evs = []
                    for b in range(g * GROUP, (g + 1) * GROUP):
                        agg = psB.tile([128, F], DT_F32, tag="agg", name="agg")
                        started = False
                        for q in range(4):
                            gt = gts[(hh, g, q)]
                            tb = int(tcnt_bq[b, q])
                            for tt in range(0, tb, 8):
                                k = min(8, tb - tt)
                                tau = int(tileof[g, q]) + tloc[q] + tt
                                oh = ohp.tile(
                                    [128, 8 * 128], DT_BF16, tag="oh", name="oh"
                                )
                                nc.vector.tensor_tensor(
                                    out=oh[:, : k * 128].rearrange(
                                        "p (t d) -> p t d", d=128
                                    ),
                                    in0=dstls[:, tau : tau + k].to_broadcast(
                                        [128, k, 128]
                                    ),
                                    in1=iotas[:, : k * 128].rearrange(
                                        "p (t d) -> p t d", d=128
                                    ),
                                    op=mybir.AluOpType.is_equal,
                                )
                                for j in range(k):
                                    ti = tloc[q] + tt + j
                                    last = q == 3 and tt + j == tb - 1
                                    nc.tensor.matmul(
                                        agg[:],
                                        oh[:, j * 128 : (j + 1) * 128],
                                        gt[:, ti * F2 : ti * F2 + F],
                                        start=not started,
                                        stop=last,
                                    )
                                    started = True
                            tloc[q] += tb
                        # evict PSUM promptly on the Scalar engine,
                        # folding in the dinv scale
                        t1e = epi.tile(
                            [128, F], DT_BF16, tag="t1e", name="t1e", bufs=8
                        )
                        nc.scalar.activation(
                            t1e[:], agg[:], COPY, scale=dinvs[:, b : b + 1]
                        )
                        evs.append((b, t1e))
                    for q in range(4):
                        del gts[(hh, g, q)]
                    return evs

                def flush_one():
                    hh, evs = pending.pop(0)
                    for b, te in evs:
                        EPIS[hh](b, te, epi, psE)

                # prologue: pre-issue hop1 gathers queue-major so each queue
                # starts as soon as its AllGather chunk lands
                npre = min(PRE, NG)
                for q in range(4):
                    if q > 0:
                        ag_chunk(cc1_in, t1, q)
                    for g in range(npre):
                        issue_gather(0, g, q, prolog=True)

                TOT = 2 * NG
                stash_q3 = []
                q3_open = False
                for G in range(TOT):
                    GP = G + npre
                    if GP < TOT:
                        hh, gg = divmod(GP, NG)
                        if hh == 1 and not q3_open:
                            for q in range(3):
                                issue_gather(1, gg, q)
                            stash_q3.append(gg)
                        else:
                            for q in range(4):
                                issue_gather(hh, gg, q)
                    hh, gg = divmod(G, NG)
                    pending.append((hh, issue_mm(hh, gg)))
                    if len(pending) > DELAY:
                        flush_one()
                    if gg == NG - 1 and hh == 0:
                        # drain hop1 epilogues so the chunk-3 AllGather's
                        # inputs are all emitted, then open hop2's q3
                        while pending:
                            flush_one()
                        ag_chunk(cc2_in, t2, 3)
                        q3_open = True
                        for sg_ in stash_q3:
                            issue_gather(1, sg_, 3)
                while pending:
                    flush_one()

    nc.compile()
    split_waits(nc)
    return nc


def kernel(in_feat, src, dst, W1, b1, W2, b2, W3, b3, W4, b4):
    global LAST_EXEC_NS
    in_feat = np.asarray(in_feat, dtype=np.float32)
    meta = _preprocess(src, dst)
    nc = _build(meta)

    dinv, perm = meta["dinv"], meta["perm"]
    W1 = np.asarray(W1, np.float32)
    W2 = np.asarray(W2, np.float32)
    W3 = np.asarray(W3, np.float32)
    W4 = np.asarray(W4, np.float32)
    b1v = np.asarray(b1, np.float32).reshape(F, 1)
    b2v = np.asarray(b2, np.float32).reshape(F, 1)
    b3v = np.asarray(b3, np.float32).reshape(F, 1)
    b4v = np.asarray(b4, np.float32).reshape(C, 1)
    w3p = np.zeros((F, 3 * F), np.float32)
    for j in range(3):
        acc = np.zeros((F, F), np.float32)
        for i in range(3):
            acc += THETAS[i, j] * W3[i * F : (i + 1) * F, :]
        w3p[:, j * F : (j + 1) * F] = acc

    ident = np.eye(128, dtype=np.float32)
    iota4 = np.tile(np.arange(128, dtype=np.float32), (128, 8)).astype(
        ml_dtypes.bfloat16
    )

    in_maps = []
    for c in range(W):
        pm = perm[c]
        real = pm >= 0
        xTc = np.zeros((R, F), np.float32)
        xTc[real] = in_feat[pm[real]]
        dv = np.zeros(R, np.float32)
        dv[real] = dinv[pm[real]]
        in_maps.append(
            {
                "xT": np.ascontiguousarray(xTc.T).astype(ml_dtypes.bfloat16),
                "dinv": np.ascontiguousarray(
                    dv.reshape(NB, 128).T
                ).astype(np.float32),
                "w1": W1.astype(ml_dtypes.bfloat16),
                "w2": W2.astype(ml_dtypes.bfloat16),
                "w3p": w3p.astype(ml_dtypes.bfloat16),
                "w4": W4.astype(ml_dtypes.bfloat16),
                "b1": b1v,
                "b2": b2v,
                "b3": b3v,
                "b4": b4v,
                "ident": ident,
                "identb": ident.astype(ml_dtypes.bfloat16),
                "iota4": iota4,
                "idx": meta["idx16"][c],
                "dstl": np.ascontiguousarray(meta["dstl"][c]),
            }
        )

    res = run_bass_kernel_spmd(nc, in_maps, core_ids=list(range(W)), trace=_TRACE)
    LAST_EXEC_NS = res.exec_time_ns

    out = np.empty((N, C), dtype=np.float32)
    for c in range(W):
        oT = res.results[c]["outT"]  # [NB, C, 128]
        om = np.transpose(oT, (0, 2, 1)).reshape(R, C)  # processed order
        pm = perm[c]
        real = pm >= 0
        out[pm[real]] = om[real]
    return out


# revision 14
# speedup vs baseline: 1.0049x; 1.0049x over previous
"""BWGNN (Bernstein-basis spectral GNN) on 8 Trainium2 NeuronCores.

Math (equivalent to the reference):
    h  = relu(relu(X W1 + b1) W2 + b2)
    L f = f - D^-1/2 A D^-1/2 f        (A via segment-sum of src->dst edges)
    g1 = L h ; g2 = L g1
    out = relu([h|g1|g2] @ W3' + b3) @ W4 + b4
W3' folds the constant Bernstein theta coefficients into W3 (a
compile-time linear reparam of the concat-of-filters formulation).

Distribution: nodes sharded 8 ways (12500/core, padded to 12544 = 98
buckets x 128, degree-sorted within core). Per Laplacian hop:
  - tables of f*dinv (bf16 rows duplicated to 256B) are AllGathered in
    4 source-position chunks so every core holds all rows; each chunk
    is one int16-addressable gather window,
  - each core gathers its in-edge messages with dma_gather (int16 idx,
    one window per SWDGE queue, 49 groups of 2 dst buckets),
  - segment-sum per 128-dst-node bucket via one-hot matmuls into PSUM.
The schedule keeps the 4 SWDGE queues (the ~10ns/descriptor critical
resource) continuously busy: gathers are pre-issued 6 groups deep per
queue, PSUM eviction happens on the Scalar engine (activation COPY with
a dinv scale), and all remaining epilogue work is issued 2 groups late
so the in-order Vector/Tensor queues never stall ahead of independent
one-hot builds / matmuls.
Dense layers are data-parallel per bucket, feature-major, bf16 matmuls
with fp32 PSUM/pointwise.
"""
import os
import sys
import types

import numpy as np
import ml_dtypes

import concourse.bacc as bacc
import concourse.bass as bass
import concourse.mybir as mybir
import concourse.tile as tile
from concourse.bass_utils import run_bass_kernel_spmd


# --- antenv.axon_hooks shim (the agent image lacks it; needed only when
# NTFF tracing is requested) ---------------------------------------------
def _install_axon_shim():
    if "antenv.axon_hooks" in sys.modules:
        return
    state = {"hook": None}
    mod = types.ModuleType("antenv.axon_hooks")
    mod.set_axon_ntff_profile_hook = lambda h: state.__setitem__("hook", h)
    mod.get_axon_ntff_profile_hook = lambda: state["hook"]
    sys.modules["antenv.axon_hooks"] = mod
    try:
        import antenv

        antenv.axon_hooks = mod
    except Exception:
        pass
    try:
        from trn_agent_boot.trn_boot import _ntff_profile_via_ctypes

        h = _ntff_profile_via_ctypes("/opt/axon/libaxon_pjrt.so")
        if h is not None:
            mod.set_axon_ntff_profile_hook(h)
    except Exception:
        pass


_install_axon_shim()


# --- BIR fixup: this walrus build rejects >1 sync wait per instruction;
# move excess waits onto preceding InstNoOp carriers (same engine, so
# semantics are unchanged) ------------------------------------------------
def split_waits(nc, max_waits=1):
    for f in nc.m.functions:
        for blk in f.blocks:
            new_insts = []
            for inst in blk.instructions:
                si = inst.sync_info
                if si is not None and len(si.on_wait) > max_waits:
                    waits = list(si.on_wait)
                    extra, keep = waits[:-max_waits], waits[-max_waits:]
                    for i in range(0, len(extra), max_waits):
                        nop = mybir.InstNoOp(
                            name=nc.get_next_instruction_name(), ins=[], outs=[]
                        )
                        nop.engine = inst.engine
                        nop.sync_info = mybir.SyncInfo(
                            on_wait=extra[i : i + max_waits], on_update=[]
                        )
                        nc.register_instruction(nop)
                        new_insts.append(nop)
                    si.on_wait = keep
                new_insts.append(inst)
            blk.instructions[:] = new_insts

N = 100_000
E = 3_200_000
F = 64
C = 2
THETAS = np.array([[3.0, -3.0, 0.75], [0.0, 3.0, -1.5], [0.0, 0.0, 0.75]])
W = 8
RPC = 12500
R = 12544                  # 98 buckets x 128
NB = 98
GROUP = 2                  # dst buckets per gather group
NG = NB // GROUP           # 49 groups
NQUEUE = 4
F2 = 2 * F                 # duplicated bf16 row = 256B

# 4 source-position chunks (for chunked AllGather + int16 gather windows)
CHB = [25, 25, 24, 24]                      # buckets per chunk
CHR = [b * 128 for b in CHB]                # rows per chunk
CHO = [0, 3200, 6400, 9472]                 # row offsets
WS = [8 * r for r in CHR]                   # gather window sizes (all < 32768)
CH_LAST_BUCKET = [24, 49, 73, 97]           # last dst bucket index per chunk
PRE = 4                                     # gather pre-issue depth (groups)
DELAY = 2                                   # epilogue flush lag (groups)

DT_BF16 = mybir.dt.bfloat16
DT_F32 = mybir.dt.float32
DT_I16 = mybir.dt.int16

LAST_EXEC_NS = None
_TRACE = os.environ.get("BWGNN_TRACE", "0") == "1"


def _preprocess(src, dst):
    src = np.asarray(src).astype(np.int64).ravel()
    dst = np.asarray(dst).astype(np.int64).ravel()
    deg = np.bincount(dst, minlength=N)
    dinv = (np.clip(deg, 1, None).astype(np.float64) ** -0.5).astype(np.float32)

    pos = np.empty(N, dtype=np.int64)
    perm = np.full((W, R), -1, dtype=np.int64)
    for c in range(W):
        g0 = c * RPC
        order = np.argsort(-deg[g0 : g0 + RPC], kind="stable")
        perm[c, :RPC] = g0 + order
        pos[g0 + order] = np.arange(RPC)

    e_c = dst // RPC
    e_pos = pos[dst]
    e_b = e_pos // 128
    e_p = (e_pos % 128).astype(np.float32)
    e_g = e_b // GROUP
    # chunked-AllGather table layout: window q = chunk of src position;
    # offset = srccore * CHR[q] + (srcpos - CHO[q])
    s_c = src // RPC
    s_pos = pos[src]
    e_q = np.searchsorted(np.array(CHO[1:] + [R]), s_pos, side="right")
    e_off = s_c * np.array(CHR)[e_q] + (s_pos - np.array(CHO)[e_q])

    key_cbq = (e_c * NB + e_b) * 4 + e_q
    cnt = np.bincount(key_cbq, minlength=W * NB * 4).reshape(W, NB, 4)
    tcnt_bq = np.maximum(1, np.ceil(cnt.max(axis=0) / 128.0).astype(np.int64))

    colof = np.zeros((NG, 4), dtype=np.int64)
    tileof = np.zeros((NG, 4), dtype=np.int64)
    nidx = np.zeros((NG, 4), dtype=np.int64)
    col = til = 0
    for g in range(NG):
        for q in range(4):
            t = int(tcnt_bq[g * GROUP : (g + 1) * GROUP, q].sum())
            colof[g, q], tileof[g, q], nidx[g, q] = col, til, t * 128
            col += t * 8
            til += t
    COLS, NTILES = col, til

    sort_key = ((e_c * NG + e_g) * 4 + e_q) * NB + e_b
    eorder = np.argsort(sort_key, kind="stable")

    idx16 = np.zeros((W, 128, COLS), dtype=np.int16)
    dstl = np.full((W, 128, NTILES), -1.0, dtype=np.float32)

    for c in range(W):
        eo = eorder[e_c[eorder] == c]
        ptr = 0
        for g in range(NG):
            for q in range(4):
                n = int(nidx[g, q])
                flat_off = np.zeros(n, dtype=np.int64)
                flat_dst = np.full(n, -1.0, dtype=np.float32)
                fpos = 0
                for b in range(g * GROUP, (g + 1) * GROUP):
                    m = int(cnt[c, b, q])
                    seg = eo[ptr : ptr + m]
                    ptr += m
                    flat_off[fpos : fpos + m] = e_off[seg]
                    flat_dst[fpos : fpos + m] = e_p[seg]
                    fpos += int(tcnt_bq[b, q]) * 128
                wrapped = flat_off.reshape(-1, 16).T.astype(np.int16)
                c0 = int(colof[g, q])
                idx16[c, :, c0 : c0 + n // 16] = np.tile(wrapped, (8, 1))
                t0 = int(tileof[g, q])
                dstl[c, :, t0 : t0 + n // 128] = flat_dst.reshape(-1, 128).T
        assert ptr == int((e_c == c).sum())

    return dict(
        dinv=dinv,
        perm=perm,
        tcnt_bq=tcnt_bq,
        colof=colof,
        tileof=tileof,
        nidx=nidx,
        COLS=COLS,
        NTILES=NTILES,
        idx16=idx16,
        dstl=dstl.astype(ml_dtypes.bfloat16),
    )


def _build(meta):
    tcnt_bq, colof, tileof, nidx = (
        meta["tcnt_bq"],
        meta["colof"],
        meta["tileof"],
        meta["nidx"],
    )
    COLS, NTILES = meta["COLS"], meta["NTILES"]

    nc = bacc.Bacc(None, target_bir_lowering=False, num_swdge_queues=NQUEUE, dynamic_dma_scratch_size=32768)

    xT = nc.declare_dram_parameter("xT", [F, R], DT_BF16, isOutput=False)
    dinv_in = nc.declare_dram_parameter("dinv", [128, NB], DT_F32, isOutput=False)
    w1 = nc.declare_dram_parameter("w1", [F, F], DT_BF16, isOutput=False)
    w2 = nc.declare_dram_parameter("w2", [F, F], DT_BF16, isOutput=False)
    w3p = nc.declare_dram_parameter("w3p", [F, 3 * F], DT_BF16, isOutput=False)
    w4 = nc.declare_dram_parameter("w4", [F, C], DT_BF16, isOutput=False)
    b1 = nc.declare_dram_parameter("b1", [F, 1], DT_F32, isOutput=False)
    b2 = nc.declare_dram_parameter("b2", [F, 1], DT_F32, isOutput=False)
    b3 = nc.declare_dram_parameter("b3", [F, 1], DT_F32, isOutput=False)
    b4 = nc.declare_dram_parameter("b4", [C, 1], DT_F32, isOutput=False)
    ident = nc.declare_dram_parameter("ident", [128, 128], DT_F32, isOutput=False)
    identb = nc.declare_dram_parameter("identb", [128, 128], DT_BF16, isOutput=False)
    iota4 = nc.declare_dram_parameter("iota4", [128, 8 * 128], DT_BF16, isOutput=False)
    idx_in = nc.declare_dram_parameter("idx", [128, COLS], DT_I16, isOutput=False)
    dstl_in = nc.declare_dram_parameter("dstl", [128, NTILES], DT_BF16, isOutput=False)
    outT = nc.declare_dram_parameter("outT", [NB, C, 128], DT_F32, isOutput=True)

    cc1_in = nc.dram_tensor("cc1_in", [R, F2], DT_BF16)
    cc2_in = nc.dram_tensor("cc2_in", [R, F2], DT_BF16)
    t1 = [
        nc.dram_tensor(f"t1_{k}", [WS[k], F2], DT_BF16, addr_space="Shared")
        for k in range(4)
    ]
    t2 = [
        nc.dram_tensor(f"t2_{k}", [WS[k], F2], DT_BF16, addr_space="Shared")
        for k in range(4)
    ]
    rg = [list(range(W))]
    RELU = mybir.ActivationFunctionType.Relu
    COPY = mybir.ActivationFunctionType.Copy

    def ag_chunk(cc_in, tabs, k):
        nc.gpsimd.collective_compute(
            "AllGather",
            mybir.AluOpType.bypass,
            replica_groups=rg,
            ins=[cc_in[CHO[k] : CHO[k] + CHR[k], :]],
            outs=[tabs[k][:]],
        )

    with tile.TileContext(nc) as tc:
        with (
            tc.tile_pool(name="const", bufs=1) as constp,
            tc.tile_pool(name="persist", bufs=1) as persist,
        ):
            def cload(nm, shape, dtype, srcap):
                t = constp.tile(shape, dtype, name=nm, tag=nm)
                nc.sync.dma_start(out=t[:], in_=srcap)
                return t

            w1s = cload("w1s", [F, F], DT_BF16, w1[:])
            w2s = cload("w2s", [F, F], DT_BF16, w2[:])
            w3s = cload("w3s", [F, 3 * F], DT_BF16, w3p[:])
            w4s = cload("w4s", [F, C], DT_BF16, w4[:])
            b1s = cload("b1s", [F, 1], DT_F32, b1[:])
            b2s = cload("b2s", [F, 1], DT_F32, b2[:])
            b3s = cload("b3s", [F, 1], DT_F32, b3[:])
            b4s = cload("b4s", [C, 1], DT_F32, b4[:])
            idents = cload("idents", [128, 128], DT_F32, ident[:])
            identbs = cload("identbs", [128, 128], DT_BF16, identb[:])
            iotas = cload("iotas", [128, 8 * 128], DT_BF16, iota4[:])
            dinvs = cload("dinvs", [128, NB], DT_F32, dinv_in[:])
            dstls = cload("dstls", [128, NTILES], DT_BF16, dstl_in[:])

            h_node = persist.tile([128, NB * F], DT_BF16)
            g1_node = persist.tile([128, NB * F], DT_BF16)

            # hop pools open BEFORE phase A so gather tiles get distinct
            # SBUF addresses (no false reuse-dependency on phase A scratch)
            ixp = tc.alloc_tile_pool(name="ixp", bufs=5)
            gpool = tc.alloc_tile_pool(name="gpl", bufs=5)
            ohp = tc.alloc_tile_pool(name="ohp", bufs=4)
            psB = tc.alloc_tile_pool(name="psB", bufs=2, space="PSUM")
            epi = tc.alloc_tile_pool(name="epi", bufs=8)

            # ---------------- phase A ----------------
            with (
                tc.tile_pool(name="pA", bufs=3) as pA,
                tc.tile_pool(name="psA", bufs=2, space="PSUM") as psA,
            ):
                for b in range(NB):
                    xt = pA.tile([F, 128], DT_BF16, tag="xt")
                    nc.sync.dma_start(out=xt[:], in_=xT[:, b * 128 : (b + 1) * 128])
                    ps1 = psA.tile([F, 128], DT_F32, tag="ps1")
                    nc.tensor.matmul(ps1[:], w1s[:], xt[:], start=True, stop=True)
                    h1 = pA.tile([F, 128], DT_BF16, tag="h1")
                    nc.vector.tensor_scalar(
                        out=h1[:], in0=ps1[:], scalar1=b1s[:], scalar2=0.0,
                        op0=mybir.AluOpType.add, op1=mybir.AluOpType.max,
                    )
                    ps2 = psA.tile([F, 128], DT_F32, tag="ps2")
                    nc.tensor.matmul(ps2[:], w2s[:], h1[:], start=True, stop=True)
                    h2T = pA.tile([F, 128], DT_F32, tag="h2T")
                    nc.scalar.activation(h2T[:], ps2[:], RELU, bias=b2s[:])
                    psT = psA.tile([128, F], DT_F32, tag="psT")
                    nc.tensor.transpose(psT[:], h2T[:], idents[:F, :F])
                    nc.vector.tensor_copy(
                        out=h_node[:, b * F : (b + 1) * F], in_=psT[:]
                    )
                    tw = pA.tile([128, F], DT_BF16, tag="tw")
                    nc.vector.tensor_scalar(
                        out=tw[:],
                        in0=psT[:],
                        scalar1=dinvs[:, b : b + 1],
                        scalar2=None,
                        op0=mybir.AluOpType.mult,
                    )
                    nc.scalar.dma_start(
                        out=cc1_in[b * 128 : (b + 1) * 128, 0:F], in_=tw[:]
                    )
                    if b == CH_LAST_BUCKET[0]:
                        ag_chunk(cc1_in, t1, 0)

            # ---------------- hops: merged cross-hop pipeline ----------------
            def epi_B(b, t1e, epi, psE):
                # g1 = h - agg*dinv   (Vector; inputs ready - issued late)
                nc.vector.tensor_tensor(
                    out=g1_node[:, b * F : (b + 1) * F],
                    in0=h_node[:, b * F : (b + 1) * F],
                    in1=t1e[:],
                    op=mybir.AluOpType.subtract,
                )
                tw = epi.tile([128, F], DT_BF16, tag="tw2", name="tw2", bufs=4)
                nc.scalar.activation(
                    tw[:],
                    g1_node[:, b * F : (b + 1) * F],
                    COPY,
                    scale=dinvs[:, b : b + 1],
                )
                nc.scalar.dma_start(
                    out=cc2_in[b * 128 : (b + 1) * 128, 0:F], in_=tw[:]
                )
                for k in range(3):
                    if b == CH_LAST_BUCKET[k]:
                        ag_chunk(cc2_in, t2, k)

            def epi_C(b, t2e, epi, psE):
                g2n = epi.tile([128, F], DT_BF16, tag="g2n", name="g2n", bufs=4)
                nc.vector.tensor_tensor(
                    out=g2n[:],
                    in0=g1_node[:, b * F : (b + 1) * F],
                    in1=t2e[:],
                    op=mybir.AluOpType.subtract,
                )
                psZ = psE.tile([F, 128], DT_F32, tag="psZ")
                pst = psE.tile([F, 128], DT_BF16, tag="pst")
                fT = epi.tile([F, 128], DT_BF16, tag="fT", bufs=2)
                nc.tensor.transpose(
                    pst[:], h_node[:, b * F : (b + 1) * F], identbs[:]
                )
                nc.scalar.activation(fT[:], pst[:], COPY)
                nc.tensor.matmul(
                    psZ[:], w3s[:, 0:F], fT[:], start=True, stop=False
                )
                g1T = epi.tile([F, 128], DT_BF16, tag="g1T", bufs=2)
                nc.tensor.transpose(
                    pst[:], g1_node[:, b * F : (b + 1) * F], identbs[:]
                )
                nc.scalar.activation(g1T[:], pst[:], COPY)
                nc.tensor.matmul(
                    psZ[:], w3s[:, F : 2 * F], g1T[:], start=False, stop=False
                )
                g2T = epi.tile([F, 128], DT_BF16, tag="g2T", bufs=2)
                nc.tensor.transpose(pst[:], g2n[:], identbs[:])
                nc.scalar.activation(g2T[:], pst[:], COPY)
                nc.tensor.matmul(
                    psZ[:], w3s[:, 2 * F : 3 * F], g2T[:], start=False, stop=True
                )
                zb = epi.tile([F, 128], DT_BF16, tag="zb", bufs=2)
                nc.scalar.activation(zb[:], psZ[:], RELU, bias=b3s[:])
                psO = psE.tile([C, 128], DT_F32, tag="psO")
                nc.tensor.matmul(psO[:], w4s[:], zb[:], start=True, stop=True)
                oadd = epi.tile([C, 128], DT_F32, tag="oadd", bufs=2)
                nc.vector.tensor_scalar(
                    out=oadd[:],
                    in0=psO[:],
                    scalar1=b4s[:],
                    scalar2=None,
                    op0=mybir.AluOpType.add,
                )
                nc.scalar.dma_start(out=outT[b, :, :], in_=oadd[:])

            psE = tc.alloc_tile_pool(name="psE", bufs=2, space="PSUM")
            if True:
                gts = {}
                pending = []
                TABS = [t1, t2]
                EPIS = [epi_B, epi_C]

                def issue_gather(hh, g, q, prolog=False):
                    n = int(nidx[g, q])
                    c0 = int(colof[g, q])
                    ix = ixp.tile(
                        [128, n // 16], DT_I16, tag=f"ix{q}", name=f"ix{hh}_{g}_{q}"
                    )
                    ix_eng = nc.gpsimd if prolog else nc.sync
                    ix_eng.dma_start(out=ix[:], in_=idx_in[:, c0 : c0 + n // 16])
                    gt = gpool.tile(
                        [128, (n // 128) * F2], DT_BF16, tag=f"g{q}",
                        name=f"gt{hh}_{g}_{q}",
                    )
                    nc.gpsimd.dma_gather(
                        gt[:].rearrange("p (c f) -> p c f", f=F2),
                        TABS[hh][q][:, :],
                        ix[:],
                        num_idxs=n,
                        num_idxs_reg=n,
                        elem_size=F2,
                        single_packet=False,
                        queue_num=(q + g) % 4,
                    )
                    gts[(hh, g, q)] = gt

                def issue_mm(hh, g):
                    tloc = {q: 0 for q in range(4)}
                    evs = []
                    for b in range(g * GROUP, (g + 1) * GROUP):
                        agg = psB.tile([128, F], DT_F32, tag="agg", name="agg")
                        started = False
                        for q in range(4):
                            gt = gts[(hh, g, q)]
                            tb = int(tcnt_bq[b, q])
                            for tt in range(0, tb, 8):
                                k = min(8, tb - tt)
                                tau = int(tileof[g, q]) + tloc[q] + tt
                                oh = ohp.tile(
                                    [128, 8 * 128], DT_BF16, tag="oh", name="oh"
                                )
                                nc.vector.tensor_tensor(
                                    out=oh[:, : k * 128].rearrange(
                                        "p (t d) -> p t d", d=128
                                    ),
                                    in0=dstls[:, tau : tau + k].to_broadcast(
                                        [128, k, 128]
                                    ),
                                    in1=iotas[:, : k * 128].rearrange(
                                        "p (t d) -> p t d", d=128
                                    ),
                                    op=mybir.AluOpType.is_equal,
                                )
                                for j in range(k):
                                    ti = tloc[q] + tt + j
                                    last = q == 3 and tt + j == tb - 1
                                    nc.tensor.matmul(
                                        agg[:],
                                        oh[:, j * 128 : (j + 1) * 128],
                                        gt[:, ti * F2 : ti * F2 + F],
                                        start=not started,
                                        stop=last,
                                    )
                                    started = True
                            tloc[q] += tb
                        # evict PSUM promptly on the Scalar engine,
                        # folding in the dinv scale
                        t1e = epi.tile(
                            [128, F], DT_BF16, tag="t1e", name="t1e", bufs=8
                        )
                        nc.scalar.activation(
                            t1e[:], agg[:], COPY, scale=dinvs[:, b : b + 1]
                        )
                        evs.append((b, t1e))
                    for q in range(4):
                        del gts[(hh, g, q)]
                    return evs

                def flush_one():
                    hh, evs = pending.pop(0)
                    for b, te in evs:
                        EPIS[hh](b, te, epi, psE)

                # prologue: pre-issue hop1 gathers queue-major so each queue
                # starts as soon as its AllGather chunk lands
                npre = min(PRE, NG)
                for q in range(4):
                    if q > 0:
                        ag_chunk(cc1_in, t1, q)
                    for g in range(npre):
                        issue_gather(0, g, q, prolog=True)

                TOT = 2 * NG
                stash_q3 = []
                q3_open = False
                for G in range(TOT):
                    GP = G + npre
                    if GP < TOT:
                        hh, gg = divmod(GP, NG)
                        if hh == 1 and not q3_open:
                            for q in range(3):
                                issue_gather(1, gg, q)
                            stash_q3.append(gg)
                        else:
                            for q in range(4):
                                issue_gather(hh, gg, q)
                    hh, gg = divmod(G, NG)
                    pending.append((hh, issue_mm(hh, gg)))
                    if len(pending) > DELAY:
                        flush_one()
                    if gg == NG - 1 and hh == 0:
                        # drain hop1 epilogues so the chunk-3 AllGather's
                        # inputs are all emitted, then open hop2's q3
                        while pending:
                            flush_one()
                        ag_chunk(cc2_in, t2, 3)
                        q3_open = True
                        for sg_ in stash_q3:
                            issue_gather(1, sg_, 3)
                while pending:
                    flush_one()

            for _pool in (psE, epi, psB, ohp, gpool, ixp):
                _pool.release()

    nc.compile()
    split_waits(nc)
    return nc


def kernel(in_feat, src, dst, W1, b1, W2, b2, W3, b3, W4, b4):
    global LAST_EXEC_NS
    in_feat = np.asarray(in_feat, dtype=np.float32)
    meta = _preprocess(src, dst)
    nc = _build(meta)

    dinv, perm = meta["dinv"], meta["perm"]
    W1 = np.asarray(W1, np.float32)
    W2 = np.asarray(W2, np.float32)
    W3 = np.asarray(W3, np.float32)
    W4 = np.asarray(W4, np.float32)
    b1v = np.asarray(b1, np.float32).reshape(F, 1)
    b2v = np.asarray(b2, np.float32).reshape(F, 1)
    b3v = np.asarray(b3, np.float32).reshape(F, 1)
    b4v = np.asarray(b4, np.float32).reshape(C, 1)
    w3p = np.zeros((F, 3 * F), np.float32)
    for j in range(3):
        acc = np.zeros((F, F), np.float32)
        for i in range(3):
            acc += THETAS[i, j] * W3[i * F : (i + 1) * F, :]
        w3p[:, j * F : (j + 1) * F] = acc

    ident = np.eye(128, dtype=np.float32)
    iota4 = np.tile(np.arange(128, dtype=np.float32), (128, 8)).astype(
        ml_dtypes.bfloat16
    )

    in_maps = []
    for c in range(W):
        pm = perm[c]
        real = pm >= 0
        xTc = np.zeros((R, F), np.float32)
        xTc[real] = in_feat[pm[real]]
        dv = np.zeros(R, np.float32)
        dv[real] = dinv[pm[real]]
        in_maps.append(
            {
                "xT": np.ascontiguousarray(xTc.T).astype(ml_dtypes.bfloat16),
                "dinv": np.ascontiguousarray(
                    dv.reshape(NB, 128).T
                ).astype(np.float32),
                "w1": W1.astype(ml_dtypes.bfloat16),
                "w2": W2.astype(ml_dtypes.bfloat16),
                "w3p": w3p.astype(ml_dtypes.bfloat16),
                "w4": W4.astype(ml_dtypes.bfloat16),
                "b1": b1v,
                "b2": b2v,
                "b3": b3v,
                "b4": b4v,
                "ident": ident,
                "identb": ident.astype(ml_dtypes.bfloat16),
                "iota4": iota4,
                "idx": meta["idx16"][c],
                "dstl": np.ascontiguousarray(meta["dstl"][c]),
            }
        )

    res = run_bass_kernel_spmd(nc, in_maps, core_ids=list(range(W)), trace=_TRACE)
    LAST_EXEC_NS = res.exec_time_ns

    out = np.empty((N, C), dtype=np.float32)
    for c in range(W):
        oT = res.results[c]["outT"]  # [NB, C, 128]
        om = np.transpose(oT, (0, 2, 1)).reshape(R, C)  # processed order
        pm = perm[c]
        real = pm >= 0
        out[pm[real]] = om[real]
    return out


# revision 17
# speedup vs baseline: 1.0052x; 1.0003x over previous
"""BWGNN (Bernstein-basis spectral GNN) on 8 Trainium2 NeuronCores.

Math (equivalent to the reference):
    h  = relu(relu(X W1 + b1) W2 + b2)
    L f = f - D^-1/2 A D^-1/2 f        (A via segment-sum of src->dst edges)
    g1 = L h ; g2 = L g1
    out = relu([h|g1|g2] @ W3' + b3) @ W4 + b4
W3' folds the constant Bernstein theta coefficients into W3 (a
compile-time linear reparam of the concat-of-filters formulation).

Distribution: nodes sharded 8 ways (12500/core, padded to 12544 = 98
buckets x 128, degree-sorted within core). Per Laplacian hop:
  - tables of f*dinv (bf16 rows duplicated to 256B) are AllGathered in
    4 source-position chunks so every core holds all rows; each chunk
    is one int16-addressable gather window,
  - each core gathers its in-edge messages with dma_gather (int16 idx,
    one window per SWDGE queue, 49 groups of 2 dst buckets),
  - segment-sum per 128-dst-node bucket via one-hot matmuls into PSUM.
The schedule keeps the 4 SWDGE queues (the ~10ns/descriptor critical
resource) continuously busy: gathers are pre-issued 6 groups deep per
queue, PSUM eviction happens on the Scalar engine (activation COPY with
a dinv scale), and all remaining epilogue work is issued 2 groups late
so the in-order Vector/Tensor queues never stall ahead of independent
one-hot builds / matmuls.
Dense layers are data-parallel per bucket, feature-major, bf16 matmuls
with fp32 PSUM/pointwise.
"""
import os
import sys
import types

import numpy as np
import ml_dtypes

import concourse.bacc as bacc
import concourse.bass as bass
import concourse.mybir as mybir
import concourse.tile as tile
from concourse.bass_utils import run_bass_kernel_spmd


# --- antenv.axon_hooks shim (the agent image lacks it; needed only when
# NTFF tracing is requested) ---------------------------------------------
def _install_axon_shim():
    if "antenv.axon_hooks" in sys.modules:
        return
    state = {"hook": None}
    mod = types.ModuleType("antenv.axon_hooks")
    mod.set_axon_ntff_profile_hook = lambda h: state.__setitem__("hook", h)
    mod.get_axon_ntff_profile_hook = lambda: state["hook"]
    sys.modules["antenv.axon_hooks"] = mod
    try:
        import antenv

        antenv.axon_hooks = mod
    except Exception:
        pass
    try:
        from trn_agent_boot.trn_boot import _ntff_profile_via_ctypes

        h = _ntff_profile_via_ctypes("/opt/axon/libaxon_pjrt.so")
        if h is not None:
            mod.set_axon_ntff_profile_hook(h)
    except Exception:
        pass


_install_axon_shim()


# --- BIR fixup: this walrus build rejects >1 sync wait per instruction;
# move excess waits onto preceding InstNoOp carriers (same engine, so
# semantics are unchanged) ------------------------------------------------
def split_waits(nc, max_waits=1):
    for f in nc.m.functions:
        for blk in f.blocks:
            new_insts = []
            for inst in blk.instructions:
                si = inst.sync_info
                if si is not None and len(si.on_wait) > max_waits:
                    waits = list(si.on_wait)
                    extra, keep = waits[:-max_waits], waits[-max_waits:]
                    for i in range(0, len(extra), max_waits):
                        nop = mybir.InstNoOp(
                            name=nc.get_next_instruction_name(), ins=[], outs=[]
                        )
                        nop.engine = inst.engine
                        nop.sync_info = mybir.SyncInfo(
                            on_wait=extra[i : i + max_waits], on_update=[]
                        )
                        nc.register_instruction(nop)
                        new_insts.append(nop)
                    si.on_wait = keep
                new_insts.append(inst)
            blk.instructions[:] = new_insts

N = 100_000
E = 3_200_000
F = 64
C = 2
THETAS = np.array([[3.0, -3.0, 0.75], [0.0, 3.0, -1.5], [0.0, 0.0, 0.75]])
W = 8
RPC = 12500
R = 12544                  # 98 buckets x 128
NB = 98
GROUP = 2                  # dst buckets per gather group
NG = NB // GROUP           # 49 groups
NQUEUE = 4
F2 = 2 * F                 # duplicated bf16 row = 256B

# 4 source-position chunks (for chunked AllGather + int16 gather windows)
CHB = [25, 25, 24, 24]                      # buckets per chunk
CHR = [b * 128 for b in CHB]                # rows per chunk
CHO = [0, 3200, 6400, 9472]                 # row offsets
WS = [8 * r for r in CHR]                   # gather window sizes (all < 32768)
CH_LAST_BUCKET = [24, 49, 73, 97]           # last dst bucket index per chunk
PRE = 4                                     # gather pre-issue depth (groups)
DELAY = 2                                   # epilogue flush lag (groups)

DT_BF16 = mybir.dt.bfloat16
DT_F32 = mybir.dt.float32
DT_I16 = mybir.dt.int16

LAST_EXEC_NS = None
_TRACE = os.environ.get("BWGNN_TRACE", "0") == "1"


def _preprocess(src, dst):
    src = np.asarray(src).astype(np.int64).ravel()
    dst = np.asarray(dst).astype(np.int64).ravel()
    deg = np.bincount(dst, minlength=N)
    dinv = (np.clip(deg, 1, None).astype(np.float64) ** -0.5).astype(np.float32)

    pos = np.empty(N, dtype=np.int64)
    perm = np.full((W, R), -1, dtype=np.int64)
    for c in range(W):
        g0 = c * RPC
        order = np.argsort(-deg[g0 : g0 + RPC], kind="stable")
        perm[c, :RPC] = g0 + order
        pos[g0 + order] = np.arange(RPC)

    e_c = dst // RPC
    e_pos = pos[dst]
    e_b = e_pos // 128
    e_p = (e_pos % 128).astype(np.float32)
    e_g = e_b // GROUP
    # chunked-AllGather table layout: window q = chunk of src position;
    # offset = srccore * CHR[q] + (srcpos - CHO[q])
    s_c = src // RPC
    s_pos = pos[src]
    e_q = np.searchsorted(np.array(CHO[1:] + [R]), s_pos, side="right")
    e_off = s_c * np.array(CHR)[e_q] + (s_pos - np.array(CHO)[e_q])

    key_cbq = (e_c * NB + e_b) * 4 + e_q
    cnt = np.bincount(key_cbq, minlength=W * NB * 4).reshape(W, NB, 4)
    tcnt_bq = np.maximum(1, np.ceil(cnt.max(axis=0) / 128.0).astype(np.int64))

    colof = np.zeros((NG, 4), dtype=np.int64)
    tileof = np.zeros((NG, 4), dtype=np.int64)
    nidx = np.zeros((NG, 4), dtype=np.int64)
    col = til = 0
    for g in range(NG):
        for q in range(4):
            t = int(tcnt_bq[g * GROUP : (g + 1) * GROUP, q].sum())
            colof[g, q], tileof[g, q], nidx[g, q] = col, til, t * 128
            col += t * 8
            til += t
    COLS, NTILES = col, til

    sort_key = ((e_c * NG + e_g) * 4 + e_q) * NB + e_b
    eorder = np.argsort(sort_key, kind="stable")

    idx16 = np.zeros((W, 128, COLS), dtype=np.int16)
    dstl = np.full((W, 128, NTILES), -1.0, dtype=np.float32)

    for c in range(W):
        eo = eorder[e_c[eorder] == c]
        ptr = 0
        for g in range(NG):
            for q in range(4):
                n = int(nidx[g, q])
                flat_off = np.zeros(n, dtype=np.int64)
                flat_dst = np.full(n, -1.0, dtype=np.float32)
                fpos = 0
                for b in range(g * GROUP, (g + 1) * GROUP):
                    m = int(cnt[c, b, q])
                    seg = eo[ptr : ptr + m]
                    ptr += m
                    flat_off[fpos : fpos + m] = e_off[seg]
                    flat_dst[fpos : fpos + m] = e_p[seg]
                    fpos += int(tcnt_bq[b, q]) * 128
                wrapped = flat_off.reshape(-1, 16).T.astype(np.int16)
                c0 = int(colof[g, q])
                idx16[c, :, c0 : c0 + n // 16] = np.tile(wrapped, (8, 1))
                t0 = int(tileof[g, q])
                dstl[c, :, t0 : t0 + n // 128] = flat_dst.reshape(-1, 128).T
        assert ptr == int((e_c == c).sum())

    return dict(
        dinv=dinv,
        perm=perm,
        tcnt_bq=tcnt_bq,
        colof=colof,
        tileof=tileof,
        nidx=nidx,
        COLS=COLS,
        NTILES=NTILES,
        idx16=idx16,
        dstl=dstl.astype(ml_dtypes.bfloat16),
    )


def _build(meta):
    tcnt_bq, colof, tileof, nidx = (
        meta["tcnt_bq"],
        meta["colof"],
        meta["tileof"],
        meta["nidx"],
    )
    COLS, NTILES = meta["COLS"], meta["NTILES"]

    nc = bacc.Bacc(None, target_bir_lowering=False, num_swdge_queues=NQUEUE, dynamic_dma_scratch_size=32768)

    xT = nc.declare_dram_parameter("xT", [F, R], DT_BF16, isOutput=False)
    dinv_in = nc.declare_dram_parameter("dinv", [128, NB], DT_F32, isOutput=False)
    w1 = nc.declare_dram_parameter("w1", [F, F], DT_BF16, isOutput=False)
    w2 = nc.declare_dram_parameter("w2", [F, F], DT_BF16, isOutput=False)
    w3p = nc.declare_dram_parameter("w3p", [F, 3 * F], DT_BF16, isOutput=False)
    w4 = nc.declare_dram_parameter("w4", [F, C], DT_BF16, isOutput=False)
    b1 = nc.declare_dram_parameter("b1", [F, 1], DT_F32, isOutput=False)
    b2 = nc.declare_dram_parameter("b2", [F, 1], DT_F32, isOutput=False)
    b3 = nc.declare_dram_parameter("b3", [F, 1], DT_F32, isOutput=False)
    b4 = nc.declare_dram_parameter("b4", [C, 1], DT_F32, isOutput=False)
    ident = nc.declare_dram_parameter("ident", [128, 128], DT_F32, isOutput=False)
    identb = nc.declare_dram_parameter("identb", [128, 128], DT_BF16, isOutput=False)
    iota4 = nc.declare_dram_parameter("iota4", [128, 8 * 128], DT_BF16, isOutput=False)
    idx_in = nc.declare_dram_parameter("idx", [128, COLS], DT_I16, isOutput=False)
    dstl_in = nc.declare_dram_parameter("dstl", [128, NTILES], DT_BF16, isOutput=False)
    outT = nc.declare_dram_parameter("outT", [NB, C, 128], DT_F32, isOutput=True)

    cc1_in = nc.dram_tensor("cc1_in", [R, F2], DT_BF16)
    cc2_in = nc.dram_tensor("cc2_in", [R, F2], DT_BF16)
    t1 = [
        nc.dram_tensor(f"t1_{k}", [WS[k], F2], DT_BF16, addr_space="Shared")
        for k in range(4)
    ]
    t2 = [
        nc.dram_tensor(f"t2_{k}", [WS[k], F2], DT_BF16, addr_space="Shared")
        for k in range(4)
    ]
    rg = [list(range(W))]
    RELU = mybir.ActivationFunctionType.Relu
    COPY = mybir.ActivationFunctionType.Copy

    def ag_chunk(cc_in, tabs, k):
        nc.gpsimd.collective_compute(
            "AllGather",
            mybir.AluOpType.bypass,
            replica_groups=rg,
            ins=[cc_in[CHO[k] : CHO[k] + CHR[k], :]],
            outs=[tabs[k][:]],
        )

    with tile.TileContext(nc) as tc:
        with (
            tc.tile_pool(name="const", bufs=1) as constp,
            tc.tile_pool(name="persist", bufs=1) as persist,
        ):
            def cload(nm, shape, dtype, srcap):
                t = constp.tile(shape, dtype, name=nm, tag=nm)
                nc.sync.dma_start(out=t[:], in_=srcap)
                return t

            w1s = cload("w1s", [F, F], DT_BF16, w1[:])
            w2s = cload("w2s", [F, F], DT_BF16, w2[:])
            w3s = cload("w3s", [F, 3 * F], DT_BF16, w3p[:])
            w4s = cload("w4s", [F, C], DT_BF16, w4[:])
            b1s = cload("b1s", [F, 1], DT_F32, b1[:])
            b2s = cload("b2s", [F, 1], DT_F32, b2[:])
            b3s = cload("b3s", [F, 1], DT_F32, b3[:])
            b4s = cload("b4s", [C, 1], DT_F32, b4[:])
            idents = cload("idents", [128, 128], DT_F32, ident[:])
            identbs = cload("identbs", [128, 128], DT_BF16, identb[:])
            iotas = cload("iotas", [128, 8 * 128], DT_BF16, iota4[:])
            dinvs = cload("dinvs", [128, NB], DT_F32, dinv_in[:])
            dstls = cload("dstls", [128, NTILES], DT_BF16, dstl_in[:])

            h_node = persist.tile([128, NB * F], DT_BF16)
            g1_node = persist.tile([128, NB * F], DT_BF16)

            # hop pools open BEFORE phase A so gather tiles get distinct
            # SBUF addresses (no false reuse-dependency on phase A scratch)
            ixp = tc.alloc_tile_pool(name="ixp", bufs=5)
            gpool = tc.alloc_tile_pool(name="gpl", bufs=5)
            ohp = tc.alloc_tile_pool(name="ohp", bufs=4)
            psB = tc.alloc_tile_pool(name="psB", bufs=2, space="PSUM")
            epi = tc.alloc_tile_pool(name="epi", bufs=8)

            # ---------------- phase A ----------------
            with (
                tc.tile_pool(name="pA", bufs=3) as pA,
                tc.tile_pool(name="psA", bufs=2, space="PSUM") as psA,
            ):
                for b in range(NB):
                    xt = pA.tile([F, 128], DT_BF16, tag="xt")
                    nc.sync.dma_start(out=xt[:], in_=xT[:, b * 128 : (b + 1) * 128])
                    ps1 = psA.tile([F, 128], DT_F32, tag="ps1")
                    nc.tensor.matmul(ps1[:], w1s[:], xt[:], start=True, stop=True)
                    h1 = pA.tile([F, 128], DT_BF16, tag="h1")
                    nc.vector.tensor_scalar(
                        out=h1[:], in0=ps1[:], scalar1=b1s[:], scalar2=0.0,
                        op0=mybir.AluOpType.add, op1=mybir.AluOpType.max,
                    )
                    ps2 = psA.tile([F, 128], DT_F32, tag="ps2")
                    nc.tensor.matmul(ps2[:], w2s[:], h1[:], start=True, stop=True)
                    h2T = pA.tile([F, 128], DT_F32, tag="h2T")
                    nc.scalar.activation(h2T[:], ps2[:], RELU, bias=b2s[:])
                    psT = psA.tile([128, F], DT_F32, tag="psT")
                    nc.tensor.transpose(psT[:], h2T[:], idents[:F, :F])
                    nc.vector.tensor_copy(
                        out=h_node[:, b * F : (b + 1) * F], in_=psT[:]
                    )
                    tw = pA.tile([128, F], DT_BF16, tag="tw")
                    nc.vector.tensor_scalar(
                        out=tw[:],
                        in0=psT[:],
                        scalar1=dinvs[:, b : b + 1],
                        scalar2=None,
                        op0=mybir.AluOpType.mult,
                    )
                    nc.scalar.dma_start(
                        out=cc1_in[b * 128 : (b + 1) * 128, 0:F], in_=tw[:]
                    )
                    for _k in range(4):
                        if b == CH_LAST_BUCKET[_k]:
                            ag_chunk(cc1_in, t1, _k)

            # ---------------- hops: merged cross-hop pipeline ----------------
            def epi_B(b, t1e, epi, psE):
                # g1 = h - agg*dinv   (Vector; inputs ready - issued late)
                nc.vector.tensor_tensor(
                    out=g1_node[:, b * F : (b + 1) * F],
                    in0=h_node[:, b * F : (b + 1) * F],
                    in1=t1e[:],
                    op=mybir.AluOpType.subtract,
                )
                tw = epi.tile([128, F], DT_BF16, tag="tw2", name="tw2", bufs=4)
                nc.scalar.activation(
                    tw[:],
                    g1_node[:, b * F : (b + 1) * F],
                    COPY,
                    scale=dinvs[:, b : b + 1],
                )
                nc.scalar.dma_start(
                    out=cc2_in[b * 128 : (b + 1) * 128, 0:F], in_=tw[:]
                )
                for k in range(3):
                    if b == CH_LAST_BUCKET[k]:
                        ag_chunk(cc2_in, t2, k)

            def epi_C(b, t2e, epi, psE):
                g2n = epi.tile([128, F], DT_BF16, tag="g2n", name="g2n", bufs=4)
                nc.vector.tensor_tensor(
                    out=g2n[:],
                    in0=g1_node[:, b * F : (b + 1) * F],
                    in1=t2e[:],
                    op=mybir.AluOpType.subtract,
                )
                psZ = psE.tile([F, 128], DT_F32, tag="psZ")
                pst = psE.tile([F, 128], DT_BF16, tag="pst")
                fT = epi.tile([F, 128], DT_BF16, tag="fT", bufs=2)
                nc.tensor.transpose(
                    pst[:], h_node[:, b * F : (b + 1) * F], identbs[:]
                )
                nc.scalar.activation(fT[:], pst[:], COPY)
                nc.tensor.matmul(
                    psZ[:], w3s[:, 0:F], fT[:], start=True, stop=False
                )
                g1T = epi.tile([F, 128], DT_BF16, tag="g1T", bufs=2)
                nc.tensor.transpose(
                    pst[:], g1_node[:, b * F : (b + 1) * F], identbs[:]
                )
                nc.scalar.activation(g1T[:], pst[:], COPY)
                nc.tensor.matmul(
                    psZ[:], w3s[:, F : 2 * F], g1T[:], start=False, stop=False
                )
                g2T = epi.tile([F, 128], DT_BF16, tag="g2T", bufs=2)
                nc.tensor.transpose(pst[:], g2n[:], identbs[:])
                nc.scalar.activation(g2T[:], pst[:], COPY)
                nc.tensor.matmul(
                    psZ[:], w3s[:, 2 * F : 3 * F], g2T[:], start=False, stop=True
                )
                zb = epi.tile([F, 128], DT_BF16, tag="zb", bufs=2)
                nc.scalar.activation(zb[:], psZ[:], RELU, bias=b3s[:])
                psO = psE.tile([C, 128], DT_F32, tag="psO")
                nc.tensor.matmul(psO[:], w4s[:], zb[:], start=True, stop=True)
                oadd = epi.tile([C, 128], DT_F32, tag="oadd", bufs=2)
                nc.vector.tensor_scalar(
                    out=oadd[:],
                    in0=psO[:],
                    scalar1=b4s[:],
                    scalar2=None,
                    op0=mybir.AluOpType.add,
                )
                nc.scalar.dma_start(out=outT[b, :, :], in_=oadd[:])

            psE = tc.alloc_tile_pool(name="psE", bufs=2, space="PSUM")
            if True:
                gts = {}
                pending = []
                TABS = [t1, t2]
                EPIS = [epi_B, epi_C]

                def issue_gather(hh, g, q, prolog=False):
                    n = int(nidx[g, q])
                    c0 = int(colof[g, q])
                    ix = ixp.tile(
                        [128, n // 16], DT_I16, tag=f"ix{q}", name=f"ix{hh}_{g}_{q}"
                    )
                    ix_eng = nc.gpsimd if prolog else nc.sync
                    ix_eng.dma_start(out=ix[:], in_=idx_in[:, c0 : c0 + n // 16])
                    gt = gpool.tile(
                        [128, (n // 128) * F2], DT_BF16, tag=f"g{q}",
                        name=f"gt{hh}_{g}_{q}",
                    )
                    nc.gpsimd.dma_gather(
                        gt[:].rearrange("p (c f) -> p c f", f=F2),
                        TABS[hh][q][:, :],
                        ix[:],
                        num_idxs=n,
                        num_idxs_reg=n,
                        elem_size=F2,
                        single_packet=False,
                        queue_num=(q + g) % 4,
                    )
                    gts[(hh, g, q)] = gt

                def issue_mm(hh, g):
                    tloc = {q: 0 for q in range(4)}
                    evs = []
                    for b in range(g * GROUP, (g + 1) * GROUP):
                        agg = psB.tile([128, F], DT_F32, tag="agg", name="agg")
                        started = False
                        for q in range(4):
                            gt = gts[(hh, g, q)]
                            tb = int(tcnt_bq[b, q])
                            for tt in range(0, tb, 8):
                                k = min(8, tb - tt)
                                tau = int(tileof[g, q]) + tloc[q] + tt
                                oh = ohp.tile(
                                    [128, 8 * 128], DT_BF16, tag="oh", name="oh"
                                )
                                nc.vector.tensor_tensor(
                                    out=oh[:, : k * 128].rearrange(
                                        "p (t d) -> p t d", d=128
                                    ),
                                    in0=dstls[:, tau : tau + k].to_broadcast(
                                        [128, k, 128]
                                    ),
                                    in1=iotas[:, : k * 128].rearrange(
                                        "p (t d) -> p t d", d=128
                                    ),
                                    op=mybir.AluOpType.is_equal,
                                )
                                for j in range(k):
                                    ti = tloc[q] + tt + j
                                    last = q == 3 and tt + j == tb - 1
                                    nc.tensor.matmul(
                                        agg[:],
                                        oh[:, j * 128 : (j + 1) * 128],
                                        gt[:, ti * F2 : ti * F2 + F],
                                        start=not started,
                                        stop=last,
                                    )
                                    started = True
                            tloc[q] += tb
                        # evict PSUM promptly on the Scalar engine,
                        # folding in the dinv scale
                        t1e = epi.tile(
                            [128, F], DT_BF16, tag="t1e", name="t1e", bufs=8
                        )
                        nc.scalar.activation(
                            t1e[:], agg[:], COPY, scale=dinvs[:, b : b + 1]
                        )
                        evs.append((b, t1e))
                    for q in range(4):
                        del gts[(hh, g, q)]
                    return evs

                def flush_one():
                    hh, evs = pending.pop(0)
                    for b, te in evs:
                        EPIS[hh](b, te, epi, psE)

                # prologue: pre-issue hop1 gathers queue-major so each queue
                # starts as soon as its AllGather chunk lands
                npre = min(PRE, NG)
                for q in range(4):
                    for g in range(npre):
                        issue_gather(0, g, q, prolog=True)

                TOT = 2 * NG
                stash_q3 = []
                q3_open = False
                for G in range(TOT):
                    GP = G + npre
                    if GP < TOT:
                        hh, gg = divmod(GP, NG)
                        if hh == 1 and not q3_open:
                            for q in range(3):
                                issue_gather(1, gg, q)
                            stash_q3.append(gg)
                        else:
                            for q in range(4):
                                issue_gather(hh, gg, q)
                    hh, gg = divmod(G, NG)
                    pending.append((hh, issue_mm(hh, gg)))
                    if len(pending) > DELAY:
                        flush_one()
                    if gg == NG - 1 and hh == 0:
                        # drain hop1 epilogues so the chunk-3 AllGather's
                        # inputs are all emitted, then open hop2's q3
                        while pending:
                            flush_one()
                        ag_chunk(cc2_in, t2, 3)
                        q3_open = True
                        for sg_ in stash_q3:
                            issue_gather(1, sg_, 3)
                while pending:
                    flush_one()

            for _pool in (psE, epi, psB, ohp, gpool, ixp):
                _pool.release()

    nc.compile()
    split_waits(nc)
    return nc


def kernel(in_feat, src, dst, W1, b1, W2, b2, W3, b3, W4, b4):
    global LAST_EXEC_NS
    in_feat = np.asarray(in_feat, dtype=np.float32)
    meta = _preprocess(src, dst)
    nc = _build(meta)

    dinv, perm = meta["dinv"], meta["perm"]
    W1 = np.asarray(W1, np.float32)
    W2 = np.asarray(W2, np.float32)
    W3 = np.asarray(W3, np.float32)
    W4 = np.asarray(W4, np.float32)
    b1v = np.asarray(b1, np.float32).reshape(F, 1)
    b2v = np.asarray(b2, np.float32).reshape(F, 1)
    b3v = np.asarray(b3, np.float32).reshape(F, 1)
    b4v = np.asarray(b4, np.float32).reshape(C, 1)
    w3p = np.zeros((F, 3 * F), np.float32)
    for j in range(3):
        acc = np.zeros((F, F), np.float32)
        for i in range(3):
            acc += THETAS[i, j] * W3[i * F : (i + 1) * F, :]
        w3p[:, j * F : (j + 1) * F] = acc

    ident = np.eye(128, dtype=np.float32)
    iota4 = np.tile(np.arange(128, dtype=np.float32), (128, 8)).astype(
        ml_dtypes.bfloat16
    )

    in_maps = []
    for c in range(W):
        pm = perm[c]
        real = pm >= 0
        xTc = np.zeros((R, F), np.float32)
        xTc[real] = in_feat[pm[real]]
        dv = np.zeros(R, np.float32)
        dv[real] = dinv[pm[real]]
        in_maps.append(
            {
                "xT": np.ascontiguousarray(xTc.T).astype(ml_dtypes.bfloat16),
                "dinv": np.ascontiguousarray(
                    dv.reshape(NB, 128).T
                ).astype(np.float32),
                "w1": W1.astype(ml_dtypes.bfloat16),
                "w2": W2.astype(ml_dtypes.bfloat16),
                "w3p": w3p.astype(ml_dtypes.bfloat16),
                "w4": W4.astype(ml_dtypes.bfloat16),
                "b1": b1v,
                "b2": b2v,
                "b3": b3v,
                "b4": b4v,
                "ident": ident,
                "identb": ident.astype(ml_dtypes.bfloat16),
                "iota4": iota4,
                "idx": meta["idx16"][c],
                "dstl": np.ascontiguousarray(meta["dstl"][c]),
            }
        )

    res = run_bass_kernel_spmd(nc, in_maps, core_ids=list(range(W)), trace=_TRACE)
    LAST_EXEC_NS = res.exec_time_ns

    out = np.empty((N, C), dtype=np.float32)
    for c in range(W):
        oT = res.results[c]["outT"]  # [NB, C, 128]
        om = np.transpose(oT, (0, 2, 1)).reshape(R, C)  # processed order
        pm = perm[c]
        real = pm >= 0
        out[pm[real]] = om[real]
    return out
